# revision 2
# baseline (speedup 1.0000x reference)
"""GateRecurrent2dnoind (horizontal, forward) Trainium2 kernel.

Semantics (matching the reference):
  G1u, G2u = bilinear 2x upsample (half-pixel) of G1, G2 to (256, 256)
  g1x = G1u * X
  o = g1x; repeat 128x: o = g1x + G2u * shift_right_w(o)   (left edge replicated)

The 128 Jacobi passes are computed as ONE sequential scan along W:
  s[x] = g1x[x] + G2u[x] * s[x-1]
with an exact depth-128 window emulation:
  - boundary init: s[-1] = b0 * sum_{m=0}^{127} a0^m   (a0 = G2u[...,0], b0 = g1x[...,0])
  - window correction: data1[x] -= q[x] for x in 1..K, where
      q[x] = b0 * a0^(129-x) * prod_{i=1..x} G2u[...,i]
    computed by a second scan q[x] = (G2u[x]/a0) * q[x-1], q-init = b0*a0^129,
    floored to 0 when a0 < 0.5 (correction < 0.5^65 there, far below f32).
    The main scan then propagates q into exactly the missing window terms.

Sharding: batch b -> core b (8 batches, 8 cores). Per core: [64, 256, 256].
Layout: H on partitions (even/odd output-row parities as separate tiles, so the
2x H-upsample is two 128x128 matmuls), W and a 4-channel block on the free dim.
"""

import contextlib

import numpy as np

import concourse.bacc as bacc
import concourse.bass as bass
import concourse.mybir as mybir
import concourse.tile as tile
from concourse.bass_utils import run_bass_kernel_spmd

f32 = mybir.dt.float32
Alu = mybir.AluOpType

NCORES = 8
C = 64          # channels per core
H = 256
W = 256
HG = 128        # G input h/w
B = 4           # channels per block
NBLK = C // B
K = 64          # correction columns


def _upsample_mats():
    """lhsT [k=in_row, m=out_row] for the H-upsample matmuls, scaled by 0.25.

    even rows: out[m] = 0.25*in[m-1] + 0.75*in[m]   (m=0 clamps to in[0])
    odd rows:  out[m] = 0.75*in[m] + 0.25*in[m+1]   (m=127 clamps to in[127])
    """
    ue = np.zeros((HG, HG), np.float32)
    uo = np.zeros((HG, HG), np.float32)
    for m in range(HG):
        ue[m, m] += 0.25 * 0.75
        ue[max(m - 1, 0), m] += 0.25 * 0.25
        uo[m, m] += 0.25 * 0.75
        uo[min(m + 1, HG - 1), m] += 0.25 * 0.25
    return ue, uo


def _emit(nc, pools, ue, uo, dram):
    pcp, pcps, ginp, psp, hupp, gupp, xinp, datp, qtp, outp = pools
    Xd, G1d, G2d, Od = dram

    # ---- per-core boundary precompute (batched over all channels) ------
    g2c0 = pcp.tile([HG, C], f32, tag="g2c0")
    nc.sync.dma_start(g2c0[:], G2d[:, :, 0].transpose([1, 0]))
    coefs = {}
    for par, u in (("e", ue), ("o", uo)):
        ps = pcps.tile([HG, C], f32, tag="pcps")
        nc.tensor.matmul(ps[:], u[:], g2c0[:], start=True, stop=True)
        a0 = pcp.tile([HG, C], f32, tag=f"a0{par}")
        nc.vector.tensor_scalar_mul(a0[:], ps[:], 4.0)
        # geo = sum_{m=0}^{127} a0^m = prod_k (1 + a0^(2^k)), k=0..6
        acc = pcp.tile([HG, C], f32, tag=f"acc{par}")
        p = pcp.tile([HG, C], f32, tag=f"p{par}")
        t = pcp.tile([HG, C], f32, tag=f"t{par}")
        nc.vector.tensor_scalar_add(acc[:], a0[:], 1.0)
        nc.vector.tensor_tensor(p[:], a0[:], a0[:], Alu.mult)
        for _ in range(5):
            nc.vector.tensor_scalar_add(t[:], p[:], 1.0)
            nc.vector.tensor_tensor(acc[:], acc[:], t[:], Alu.mult)
            nc.vector.tensor_tensor(p[:], p[:], p[:], Alu.mult)
        nc.vector.tensor_scalar_add(t[:], p[:], 1.0)
        nc.vector.tensor_tensor(acc[:], acc[:], t[:], Alu.mult)
        a128 = pcp.tile([HG, C], f32, tag=f"a128{par}")
        nc.vector.tensor_tensor(a128[:], p[:], p[:], Alu.mult)
        # s0coef = 1 + a0*geo  (s[0] = b0*s0coef)
        s0c = pcp.tile([HG, C], f32, tag=f"s0c{par}")
        nc.vector.tensor_tensor(t[:], a0[:], acc[:], Alu.mult)
        nc.vector.tensor_scalar_add(s0c[:], t[:], 1.0)
        # qcoef = mask(a0>=0.5) * a128 * a0   (q-init = b0*qcoef)
        mask = pcp.tile([HG, C], f32, tag=f"mask{par}")
        nc.vector.tensor_scalar(mask[:], a0[:], 0.5, None, Alu.is_ge)
        rec = pcp.tile([HG, C], f32, tag=f"rec{par}")
        nc.vector.tensor_scalar_max(t[:], a0[:], 0.5)
        nc.vector.reciprocal(rec[:], t[:])
        qc = pcp.tile([HG, C], f32, tag=f"qc{par}")
        nc.vector.tensor_tensor(qc[:], mask[:], a128[:], Alu.mult)
        nc.vector.tensor_tensor(qc[:], qc[:], a0[:], Alu.mult)
        coefs[par] = (s0c, qc, rec)

    # ---- main loop -----------------------------------------------------
    for blk in range(NBLK):
        c0 = blk * B
        g1b = ginp.tile([HG, B * HG], f32, tag="g1b")
        g2b = ginp.tile([HG, B * HG], f32, tag="g2b")
        nc.sync.dma_start(
            g1b[:].rearrange("p (c w) -> p c w", c=B),
            G1d[c0:c0 + B, :, :].transpose([1, 0, 2]))
        nc.sync.dma_start(
            g2b[:].rearrange("p (c w) -> p c w", c=B),
            G2d[c0:c0 + B, :, :].transpose([1, 0, 2]))
        for par, u in (("e", ue), ("o", uo)):
            s0c, qc, rec = coefs[par]
            pstart = 0 if par == "e" else 1
            # H-upsample (PE): A = 0.25 * Hup  [128, (B,128)]
            a1 = psp.tile([HG, B * HG], f32, tag="a1")
            a2 = psp.tile([HG, B * HG], f32, tag="a2")
            nc.tensor.matmul(a1[:], u[:], g1b[:], start=True, stop=True)
            nc.tensor.matmul(a2[:], u[:], g2b[:], start=True, stop=True)
            c1 = hupp.tile([HG, B * HG], f32, tag="c1")
            c2 = hupp.tile([HG, B * HG], f32, tag="c2")
            c2x3 = hupp.tile([HG, B * HG], f32, tag="c2x3")
            nc.scalar.copy(c1[:], a1[:])
            nc.scalar.copy(c2[:], a2[:])
            nc.scalar.mul(c2x3[:], a2[:], 3.0)

            # W-upsample: out[2j] = 3*q[j] + q[j-1]; out[2j+1] = 3*q[j] + q[j+1]
            g1u = gupp.tile([HG, B * W], f32, tag="g1u")
            g2u = gupp.tile([HG, B * W], f32, tag="g2u")
            c1r = c1[:].rearrange("p (c w) -> p c w", c=B)
            c2r = c2[:].rearrange("p (c w) -> p c w", c=B)
            c23r = c2x3[:].rearrange("p (c w) -> p c w", c=B)
            g1r = g1u[:].rearrange("p (c w) -> p c w", c=B)
            g2r = g2u[:].rearrange("p (c w) -> p c w", c=B)
            # G1 on vector (scalar_tensor_tensor: (q*3) + q_shift)
            nc.vector.scalar_tensor_tensor(
                g1r[:, :, 2:W:2], c1r[:, :, 1:HG], 3.0,
                c1r[:, :, 0:HG - 1], Alu.mult, Alu.add)
            nc.vector.scalar_tensor_tensor(
                g1r[:, :, 1:W - 1:2], c1r[:, :, 0:HG - 1], 3.0,
                c1r[:, :, 1:HG], Alu.mult, Alu.add)
            nc.vector.scalar_tensor_tensor(
                g1r[:, :, 0:1], c1r[:, :, 0:1], 3.0,
                c1r[:, :, 0:1], Alu.mult, Alu.add)
            nc.vector.scalar_tensor_tensor(
                g1r[:, :, W - 1:W], c1r[:, :, HG - 1:HG], 3.0,
                c1r[:, :, HG - 1:HG], Alu.mult, Alu.add)
            # G2 on gpsimd (tensor_tensor only: out = 3q + q_shift);
            # col 0 must be 0 (scan re-init at channel seam)
            nc.gpsimd.tensor_tensor(
                g2r[:, :, 2:W:2], c23r[:, :, 1:HG],
                c2r[:, :, 0:HG - 1], Alu.add)
            nc.gpsimd.tensor_tensor(
                g2r[:, :, 1:W - 1:2], c23r[:, :, 0:HG - 1],
                c2r[:, :, 1:HG], Alu.add)
            nc.gpsimd.memset(g2r[:, :, 0:1], 0.0)
            nc.gpsimd.tensor_tensor(
                g2r[:, :, W - 1:W], c23r[:, :, HG - 1:HG],
                c2r[:, :, HG - 1:HG], Alu.add)

            # X block for this parity
            xb = xinp.tile([HG, B * W], f32, tag="xb")
            nc.sync.dma_start(
                xb[:].rearrange("p (c w) -> p c w", c=B),
                Xd[c0:c0 + B, pstart:H:2, :].transpose([1, 0, 2]))

            # data1 = g1x
            d = datp.tile([HG, B * W], f32, tag="d")
            nc.vector.tensor_tensor(d[:], g1u[:], xb[:], Alu.mult)
            dr = d[:].rearrange("p (c w) -> p c w", c=B)

            # window correction scan over cols 1..K
            recb = qtp.tile([HG, B * K], f32, tag="recb")
            recb_r = recb[:].rearrange("p (c w) -> p c w", c=B)
            nc.vector.tensor_copy(
                recb_r[:, :, :],
                rec[:, c0:c0 + B].unsqueeze(-1).to_broadcast([HG, B, K]))
            qd = qtp.tile([HG, B * (K + 1)], f32, tag="qd")
            qz = qtp.tile([HG, B * (K + 1)], f32, tag="qz")
            qo = qtp.tile([HG, B * (K + 1)], f32, tag="qo")
            qdr = qd[:].rearrange("p (c w) -> p c w", c=B)
            qzr = qz[:].rearrange("p (c w) -> p c w", c=B)
            qor = qo[:].rearrange("p (c w) -> p c w", c=B)
            nc.gpsimd.memset(qdr[:, :, 0:1], 0.0)
            nc.gpsimd.tensor_tensor(
                qdr[:, :, 1:K + 1], g2r[:, :, 1:K + 1], recb_r[:, :, :],
                Alu.mult)
            nc.gpsimd.memset(qz[:], 0.0)
            # spacer data1 = b0*qcoef; b0 = d[:, (c, 0)] (pre-overwrite)
            nc.vector.tensor_tensor(
                qzr[:, :, 0:1], dr[:, :, 0:1],
                qc[:, c0:c0 + B].unsqueeze(-1), Alu.mult)
            # d col0 = b0 * s0coef (in place, after qz spacer read)
            nc.vector.tensor_tensor(
                dr[:, :, 0:1], dr[:, :, 0:1],
                s0c[:, c0:c0 + B].unsqueeze(-1), Alu.mult)
            nc.vector.tensor_tensor_scan(
                qo[:], qd[:], qz[:], 0.0, Alu.mult, Alu.add)
            nc.gpsimd.tensor_tensor(
                dr[:, :, 1:K + 1], dr[:, :, 1:K + 1], qor[:, :, 1:K + 1],
                Alu.subtract)

            # main scan
            ot = outp.tile([HG, B * W], f32, tag="ot")
            nc.vector.tensor_tensor_scan(
                ot[:], g2u[:], d[:], 0.0, Alu.mult, Alu.add)
            nc.sync.dma_start(
                Od[c0:c0 + B, pstart:H:2, :].transpose([1, 0, 2]),
                ot[:].rearrange("p (c w) -> p c w", c=B))


def build(loop_n=None):
    nc = bacc.Bacc("TRN2", target_bir_lowering=False, debug=False,
                   num_devices=NCORES)
    Xd = nc.dram_tensor("X", [C, H, W], f32, kind="ExternalInput")
    G1d = nc.dram_tensor("G1", [C, HG, HG], f32, kind="ExternalInput")
    G2d = nc.dram_tensor("G2", [C, HG, HG], f32, kind="ExternalInput")
    UEd = nc.dram_tensor("UE", [HG, HG], f32, kind="ExternalInput")
    UOd = nc.dram_tensor("UO", [HG, HG], f32, kind="ExternalInput")
    Od = nc.dram_tensor("O", [C, H, W], f32, kind="ExternalOutput")

    with tile.TileContext(nc) as tc:
        with (
            tc.tile_pool(name="const", bufs=1) as constp,
            tc.tile_pool(name="pc", bufs=1) as pcp,
            tc.tile_pool(name="pcps", bufs=2, space="PSUM") as pcps,
            tc.tile_pool(name="gin", bufs=3) as ginp,
            tc.tile_pool(name="ps", bufs=2, space="PSUM") as psp,
            tc.tile_pool(name="hupc", bufs=3) as hupp,
            tc.tile_pool(name="gup", bufs=3) as gupp,
            tc.tile_pool(name="xin", bufs=3) as xinp,
            tc.tile_pool(name="dat", bufs=3) as datp,
            tc.tile_pool(name="qt", bufs=3) as qtp,
            tc.tile_pool(name="out", bufs=3) as outp,
        ):
            ue = constp.tile([HG, HG], f32, tag="ue")
            uo = constp.tile([HG, HG], f32, tag="uo")
            nc.sync.dma_start(ue[:], UEd[:])
            nc.sync.dma_start(uo[:], UOd[:])

            pools = (pcp, pcps, ginp, psp, hupp, gupp, xinp, datp, qtp, outp)
            dram = (Xd, G1d, G2d, Od)
            if loop_n:
                with tc.For_i(0, loop_n, 1):
                    _emit(nc, pools, ue, uo, dram)
            else:
                _emit(nc, pools, ue, uo, dram)

    nc.compile()
    return nc


_NC = None


def kernel(X, G1, G2, G3=None, **_):
    global _NC
    if _NC is None:
        _NC = build()
    ue, uo = _upsample_mats()
    in_maps = [
        {"X": np.ascontiguousarray(X[k]), "G1": np.ascontiguousarray(G1[k]),
         "G2": np.ascontiguousarray(G2[k]), "UE": ue, "UO": uo}
        for k in range(NCORES)
    ]
    res = run_bass_kernel_spmd(_NC, in_maps, list(range(NCORES)))
    kernel.last_result = res
    out = np.stack([res.results[k]["O"] for k in range(NCORES)])
    return out.astype(np.float32, copy=False)



# revision 3
# speedup vs baseline: 1.3128x; 1.3128x over previous
"""GateRecurrent2dnoind (horizontal, forward) Trainium2 kernel, v2.

Semantics (matching the reference):
  G1u, G2u = bilinear 2x upsample (half-pixel) of G1, G2 to (256, 256)
  g1x = G1u * X
  o = g1x; repeat 128x: o = g1x + G2u * shift_right_w(o)   (left edge replicated)

The 128 Jacobi passes collapse into ONE sequential scan along W with an exact
depth-128 window emulation (boundary geometric series + per-column window
corrections propagated by the scan; see _emit for the coefficient math).

v2 design (vs the fp32 baseline):
  - fp16 end-to-end for the big tensors (X, G in DRAM, d, g1u, output).
    Measured rel err 1.5e-3 vs the 2e-2 gate.
  - BOTH upsample directions run on the TensorEngine: for each PSUM bank,
    matmul#1 (weights 3U, moving AP with a stride-0 "repeat" dim) writes the
    0.75-weighted center tap to both W-parity slots; matmul#2 (weights U,
    moving AP with a (j:+1, rep:+2) window over a host-padded G tile)
    accumulates the 0.25-weighted left/right taps.  The full upsampled gate
    lands in PSUM with no vector-engine work.
  - The main scan reads g2u directly from PSUM (fp32 data0 = 2 cyc/elem; a
    16-bit data0 would run at 4 cyc/elem), data1 = d (fp16), out fp16.
  - VectorE does only: the two scans + the d = g1u*x multiply (fp16 2x mode)
    + a tiny PSUM col0 memset.  GpSimd does the small correction ops.
    ScalarE does the PSUM->SBUF cast copies.

Sharding: batch b -> core b (8 batches, 8 cores). Per core: [64, 256, 256].
"""

import numpy as np

import concourse.bacc as bacc
import concourse.mybir as mybir
import concourse.tile as tile
from concourse.ap import AP
from concourse.bass_utils import run_bass_kernel_spmd

f32 = mybir.dt.float32
f16 = mybir.dt.float16
Alu = mybir.AluOpType

NCORES = 8
C = 64          # channels per core
H = 256
W = 256
HG = 128        # G input h/w
WPAD = HG + 2   # G w + replicate pads
B = 4           # channels per block
NBLK = C // B
K = 32          # correction columns
THRESH = 0.75   # a0 mask/clamp for the correction chain


def _upsample_mats():
    """[k=in_row, m=out_row] H-upsample matrices, scaled by 0.25.

    even rows: out[m] = 0.25*in[m-1] + 0.75*in[m]   (m=0 clamps to in[0])
    odd rows:  out[m] = 0.75*in[m] + 0.25*in[m+1]   (m=127 clamps to in[127])
    """
    ue = np.zeros((HG, HG), np.float32)
    uo = np.zeros((HG, HG), np.float32)
    for m in range(HG):
        ue[m, m] += 0.25 * 0.75
        ue[max(m - 1, 0), m] += 0.25 * 0.25
        uo[m, m] += 0.25 * 0.75
        uo[min(m + 1, HG - 1), m] += 0.25 * 0.25
    return ue, uo


def _rep_ap(anchor, dims):
    """Raw AP sharing anchor's tensor/offset/partition dim, custom free dims."""
    return AP(anchor.tensor, anchor.offset, [list(anchor.ap[0])] + dims)


def _emit(nc, pools, weights, dram):
    (ginp, psp, xinp, g1sp, datp, g2kp, qop, outp, constp) = pools
    Xd, G1d, G2d, Od = dram

    # ---- per-parity boundary precompute (all channels at once) ---------
    # a0 = g2u[..., 0] = 4 * (U_par @ G2[:, :, 0]); the scan's full-history
    # geometric pile-up at the replicated left edge is corrected exactly via
    #   s[0] *= s0c = 1 + a0 * sum_{m=0}^{127} a0^m
    #   d[x] -= q[x] for x=1..K, q[x] = (b0*qc) * prod_{i=1..x}(g2u[i]*rec)
    #   with qc = mask(a0>=T) * a0^129, rec = 1/max(a0, T).
    g2c0 = constp.tile([HG, C], f16, tag="g2c0")
    nc.sync.dma_start(g2c0[:], G2d[:, :, 1].transpose([1, 0]))
    coefs = {}
    for par in ("e", "o"):
        u1 = weights[par + "1"]
        ps = psp.tile([HG, C], f32, tag="ps")
        nc.tensor.matmul(ps[:], u1[:], g2c0[:], start=True, stop=True)
        a0 = constp.tile([HG, C], f32, tag=f"a0{par}")
        nc.vector.tensor_scalar_mul(a0[:], ps[:], 4.0)
        # geo = sum_{m=0}^{127} a0^m = prod_k (1 + a0^(2^k)), k=0..6
        acc = constp.tile([HG, C], f32, tag=f"acc{par}")
        p = constp.tile([HG, C], f32, tag=f"p{par}")
        t = constp.tile([HG, C], f32, tag=f"t{par}")
        nc.vector.tensor_scalar_add(acc[:], a0[:], 1.0)
        nc.vector.tensor_tensor(p[:], a0[:], a0[:], Alu.mult)
        for _ in range(5):
            nc.vector.tensor_scalar_add(t[:], p[:], 1.0)
            nc.vector.tensor_tensor(acc[:], acc[:], t[:], Alu.mult)
            nc.vector.tensor_tensor(p[:], p[:], p[:], Alu.mult)
        nc.vector.tensor_scalar_add(t[:], p[:], 1.0)
        nc.vector.tensor_tensor(acc[:], acc[:], t[:], Alu.mult)
        a128 = constp.tile([HG, C], f32, tag=f"a128{par}")
        nc.vector.tensor_tensor(a128[:], p[:], p[:], Alu.mult)
        # s0c = 1 + a0*geo  (fp16 copy for the gpsimd col0 scale)
        s0cf = constp.tile([HG, C], f32, tag=f"s0cf{par}")
        nc.vector.tensor_tensor(t[:], a0[:], acc[:], Alu.mult)
        nc.vector.tensor_scalar_add(s0cf[:], t[:], 1.0)
        s0c = constp.tile([HG, C], f16, tag=f"s0c{par}")
        nc.vector.tensor_copy(s0c[:], s0cf[:])
        # qc = mask(a0>=T) * a0^128 * a0
        mask = constp.tile([HG, C], f32, tag=f"mask{par}")
        nc.vector.tensor_scalar(mask[:], a0[:], THRESH, None, Alu.is_ge)
        qcf = constp.tile([HG, C], f32, tag=f"qcf{par}")
        nc.vector.tensor_tensor(qcf[:], mask[:], a128[:], Alu.mult)
        nc.vector.tensor_tensor(qcf[:], qcf[:], a0[:], Alu.mult)
        qc = constp.tile([HG, C], f16, tag=f"qc{par}")
        nc.vector.tensor_copy(qc[:], qcf[:])
        # rec = 1/max(a0, T), broadcast over the K correction columns
        rec = constp.tile([HG, C], f32, tag=f"rec{par}")
        nc.vector.tensor_scalar_max(t[:], a0[:], THRESH)
        nc.vector.reciprocal(rec[:], t[:])
        recb = constp.tile([HG, C * K], f32, tag=f"recb{par}")
        nc.vector.tensor_copy(
            recb[:].rearrange("p (c k) -> p c k", c=C),
            rec[:].unsqueeze(-1).to_broadcast([HG, C, K]))
        coefs[par] = (s0c, qc, recb)

    # persistent correction tiles: qd col0 / qz cols stay zero across iters
    qd = constp.tile([HG, B * (K + 1)], f32, tag="qd")
    qz = constp.tile([HG, B * (K + 1)], f16, tag="qz")
    nc.vector.memset(qd[:], 0.0)
    nc.vector.memset(qz[:], 0.0)
    qdr = qd[:].rearrange("p (c w) -> p c w", c=B)
    qzr = qz[:].rearrange("p (c w) -> p c w", c=B)

    # ---- main loop -----------------------------------------------------
    for blk in range(NBLK):
        c0 = blk * B
        gb = ginp.tile([HG, 2 * B * WPAD], f16, tag="gb")
        gbr = gb[:].rearrange("p (t c w) -> p t c w", t=2, c=B)
        nc.sync.dma_start(gbr[:, 0], G1d[c0:c0 + B].transpose([1, 0, 2]))
        nc.sync.dma_start(gbr[:, 1], G2d[c0:c0 + B].transpose([1, 0, 2]))

        for par in ("e", "o"):
            s0c, qc, recb = coefs[par]
            u3 = weights[par + "3"]
            u1 = weights[par + "1"]
            pstart = 0 if par == "e" else 1

            # PE: H+W upsample straight into PSUM. Layout: [g1u | g2u],
            # each [128, (B,256)] fp32; one matmul pair per 2KB bank.
            ps = psp.tile([HG, 2 * B * W], f32, tag="ps")
            for t in range(2):
                for cp in range(B // 2):
                    dst = ps[:][:, (t * B + cp * 2) * W:(t * B + cp * 2 + 2) * W]
                    center = _rep_ap(gbr[:, t, cp * 2, 1:2],
                                     [[WPAD, 2], [1, HG], [0, 2]])
                    nc.tensor.matmul(dst, u3[:], center, start=True, stop=False)
                    shift = _rep_ap(gbr[:, t, cp * 2, 0:1],
                                    [[WPAD, 2], [1, HG], [2, 2]])
                    nc.tensor.matmul(dst, u1[:], shift, start=False, stop=True)
            g1u_ps = ps[:][:, 0:B * W]
            g2u_ps = ps[:][:, B * W:2 * B * W]
            g2u_r = g2u_ps.rearrange("p (c w) -> p c w", c=B)
            # channel-seam reset for the scan carry
            nc.vector.memset(g2u_r[:, :, 0:1], 0.0)

            # ScalarE: PSUM->SBUF copies (g1u cast fp16; g2u correction cols)
            g1u = g1sp.tile([HG, B * W], f16, tag="g1u")
            nc.scalar.copy(g1u[:], g1u_ps)
            g2k = g2kp.tile([HG, B * K], f32, tag="g2k")
            nc.scalar.copy(g2k[:].rearrange("p (c k) -> p c k", c=B),
                           g2u_r[:, :, 1:K + 1])

            # X block for this parity
            xb = xinp.tile([HG, B * W], f16, tag="xb")
            nc.sync.dma_start(
                xb[:].rearrange("p (c w) -> p c w", c=B),
                Xd[c0:c0 + B, pstart:H:2, :].transpose([1, 0, 2]))

            # d = g1u * x  (fp16 -> 2x DVE mode)
            d = datp.tile([HG, B * W], f16, tag="d")
            nc.vector.tensor_tensor(d[:], g1u[:], xb[:], Alu.mult)
            dr = d[:].rearrange("p (c w) -> p c w", c=B)

            # correction chain (gpsimd + one small DVE scan)
            # spacer: qz[c,0] = b0*qc  (b0 = d[c,0] pre-scale)
            nc.gpsimd.tensor_tensor(
                qzr[:, :, 0:1], dr[:, :, 0:1],
                qc[:, c0:c0 + B].unsqueeze(-1), Alu.mult)
            # d[c,0] *= s0c (in place, after the spacer read)
            nc.gpsimd.tensor_tensor(
                dr[:, :, 0:1], dr[:, :, 0:1],
                s0c[:, c0:c0 + B].unsqueeze(-1), Alu.mult)
            # qd[c,1:] = g2u[c,1:K+1] * rec
            nc.gpsimd.tensor_tensor(
                qdr[:, :, 1:K + 1],
                g2k[:].rearrange("p (c k) -> p c k", c=B),
                recb[:].rearrange("p (c k) -> p c k", c=C)[:, c0:c0 + B],
                Alu.mult)
            # q[x] via scan; subtract from d
            qo = qop.tile([HG, B * (K + 1)], f16, tag="qo")
            nc.vector.tensor_tensor_scan(
                qo[:], qd[:], qz[:], 0.0, Alu.mult, Alu.add)
            qor = qo[:].rearrange("p (c w) -> p c w", c=B)
            nc.gpsimd.tensor_tensor(
                dr[:, :, 1:K + 1], dr[:, :, 1:K + 1], qor[:, :, 1:K + 1],
                Alu.subtract)

            # main scan: s[x] = g2u[x]*s[x-1] + d[x]
            ot = outp.tile([HG, B * W], f16, tag="ot")
            nc.vector.tensor_tensor_scan(
                ot[:], g2u_ps, d[:], 0.0, Alu.mult, Alu.add)
            nc.sync.dma_start(
                Od[c0:c0 + B, pstart:H:2, :].transpose([1, 0, 2]),
                ot[:].rearrange("p (c w) -> p c w", c=B))


def build():
    nc = bacc.Bacc("TRN2", target_bir_lowering=False, debug=False,
                   num_devices=NCORES)
    Xd = nc.dram_tensor("X", [C, H, W], f16, kind="ExternalInput")
    G1d = nc.dram_tensor("G1", [C, HG, WPAD], f16, kind="ExternalInput")
    G2d = nc.dram_tensor("G2", [C, HG, WPAD], f16, kind="ExternalInput")
    Ud = {n: nc.dram_tensor(n.upper(), [HG, HG], f16, kind="ExternalInput")
          for n in ("e3", "e1", "o3", "o1")}
    Od = nc.dram_tensor("O", [C, H, W], f16, kind="ExternalOutput")

    with tile.TileContext(nc) as tc:
        with (
            tc.tile_pool(name="const", bufs=1) as constp,
            tc.tile_pool(name="gin", bufs=3) as ginp,
            tc.tile_pool(name="ps", bufs=2, space="PSUM") as psp,
            tc.tile_pool(name="xin", bufs=3) as xinp,
            tc.tile_pool(name="g1s", bufs=3) as g1sp,
            tc.tile_pool(name="dat", bufs=3) as datp,
            tc.tile_pool(name="g2k", bufs=3) as g2kp,
            tc.tile_pool(name="qo", bufs=3) as qop,
            tc.tile_pool(name="out", bufs=3) as outp,
        ):
            weights = {}
            for n in ("e3", "e1", "o3", "o1"):
                w = constp.tile([HG, HG], f16, tag=f"u{n}")
                nc.sync.dma_start(w[:], Ud[n][:])
                weights[n] = w
            pools = (ginp, psp, xinp, g1sp, datp, g2kp, qop, outp, constp)
            _emit(nc, pools, weights, (Xd, G1d, G2d, Od))

    nc.compile()
    return nc


_NC = None


def kernel(X, G1, G2, G3=None, **_):
    global _NC
    if _NC is None:
        _NC = build()
    ue, uo = _upsample_mats()
    wmats = {"E3": (3.0 * ue).astype(np.float16),
             "E1": ue.astype(np.float16),
             "O3": (3.0 * uo).astype(np.float16),
             "O1": uo.astype(np.float16)}

    def pad(G):
        return np.concatenate([G[..., :1], G, G[..., -1:]], axis=-1)

    Xh = np.ascontiguousarray(X).astype(np.float16)
    G1h = pad(np.asarray(G1)).astype(np.float16)
    G2h = pad(np.asarray(G2)).astype(np.float16)

    in_maps = [
        {"X": Xh[k], "G1": np.ascontiguousarray(G1h[k]),
         "G2": np.ascontiguousarray(G2h[k]), **wmats}
        for k in range(NCORES)
    ]
    res = run_bass_kernel_spmd(_NC, in_maps, list(range(NCORES)))
    kernel.last_result = res
    out = np.stack([res.results[k]["O"] for k in range(NCORES)])
    return out.astype(np.float32)


# revision 11
# speedup vs baseline: 1.5733x; 1.1985x over previous
"""GateRecurrent2dnoind (horizontal, forward) Trainium2 kernel, v2.

Semantics (matching the reference):
  G1u, G2u = bilinear 2x upsample (half-pixel) of G1, G2 to (256, 256)
  g1x = G1u * X
  o = g1x; repeat 128x: o = g1x + G2u * shift_right_w(o)   (left edge replicated)

The 128 Jacobi passes collapse into ONE sequential scan along W with an exact
depth-128 window emulation (boundary geometric series + per-column window
corrections propagated by the scan; see _emit for the coefficient math).

v2 design (vs the fp32 baseline):
  - fp16 end-to-end for the big tensors (X, G in DRAM, d, g1u, output).
    Measured rel err 1.5e-3 vs the 2e-2 gate.
  - BOTH upsample directions run on the TensorEngine: for each PSUM bank,
    matmul#1 (weights 3U, moving AP with a stride-0 "repeat" dim) writes the
    0.75-weighted center tap to both W-parity slots; matmul#2 (weights U,
    moving AP with a (j:+1, rep:+2) window over a host-padded G tile)
    accumulates the 0.25-weighted left/right taps.  The full upsampled gate
    lands in PSUM with no vector-engine work.
  - The main scan reads g2u directly from PSUM (fp32 data0 = 2 cyc/elem; a
    16-bit data0 would run at 4 cyc/elem), data1 = d (fp16), out fp16.
  - VectorE does only: the two scans + the d = g1u*x multiply (fp16 2x mode)
    + a tiny PSUM col0 memset.  GpSimd does the small correction ops.
    ScalarE does the PSUM->SBUF cast copies.

Sharding: batch b -> core b (8 batches, 8 cores). Per core: [64, 256, 256].
"""

import numpy as np

import concourse.bacc as bacc
import concourse.mybir as mybir
import concourse.tile as tile
from concourse.ap import AP
from concourse.bass_utils import run_bass_kernel_spmd

f32 = mybir.dt.float32
f16 = mybir.dt.float16
Alu = mybir.AluOpType

NCORES = 8
C = 64          # channels per core
H = 256
W = 256
HG = 128        # G input h/w
WPAD = HG + 2   # G w + replicate pads
B = 4           # channels per block
NBLK = C // B
K = 32          # correction columns
THRESH = 0.75   # a0 mask/clamp for the correction chain


def _upsample_mats():
    """[k=in_row, m=out_row] H-upsample matrices, scaled by 0.25.

    even rows: out[m] = 0.25*in[m-1] + 0.75*in[m]   (m=0 clamps to in[0])
    odd rows:  out[m] = 0.75*in[m] + 0.25*in[m+1]   (m=127 clamps to in[127])
    """
    ue = np.zeros((HG, HG), np.float32)
    uo = np.zeros((HG, HG), np.float32)
    for m in range(HG):
        ue[m, m] += 0.25 * 0.75
        ue[max(m - 1, 0), m] += 0.25 * 0.25
        uo[m, m] += 0.25 * 0.75
        uo[min(m + 1, HG - 1), m] += 0.25 * 0.25
    return ue, uo


def _rep_ap(anchor, dims):
    """Raw AP sharing anchor's tensor/offset/partition dim, custom free dims."""
    return AP(anchor.tensor, anchor.offset, [list(anchor.ap[0])] + dims)


def _emit(nc, pools, weights, dram):
    (ginp, psp, xinp, g1sp, datp, g2kp, qop, outp, constp) = pools
    Xd, G12d, Od = dram

    # ---- per-parity boundary precompute (all channels at once) ---------
    # a0 = g2u[..., 0] = 4 * (U_par @ G2[:, :, 0]); the scan's full-history
    # geometric pile-up at the replicated left edge is corrected exactly via
    #   s[0] *= s0c = 1 + a0 * sum_{m=0}^{127} a0^m
    #   d[x] -= q[x] for x=1..K, q[x] = (b0*qc) * prod_{i=1..x}(g2u[i]*rec)
    #   with qc = mask(a0>=T) * a0^129, rec = 1/max(a0, T).
    g2c0 = constp.tile([HG, C], f16, tag="g2c0")
    nc.sync.dma_start(g2c0[:], G12d[:, :, 1, 1].transpose([1, 0]))
    coefs = {}
    for par in ("e", "o"):
        u1 = weights[par + "1"]
        ps = psp.tile([HG, C], f32, tag="ps")
        nc.tensor.matmul(ps[:], u1[:], g2c0[:], start=True, stop=True)
        a0 = constp.tile([HG, C], f32, tag=f"a0{par}")
        nc.vector.tensor_scalar_mul(a0[:], ps[:], 4.0)
        # geo = sum_{m=0}^{127} a0^m = prod_k (1 + a0^(2^k)), k=0..6
        acc = constp.tile([HG, C], f32, tag=f"acc{par}")
        p = constp.tile([HG, C], f32, tag=f"p{par}")
        t = constp.tile([HG, C], f32, tag=f"t{par}")
        nc.vector.tensor_scalar_add(acc[:], a0[:], 1.0)
        nc.vector.tensor_tensor(p[:], a0[:], a0[:], Alu.mult)
        for _ in range(5):
            nc.vector.tensor_scalar_add(t[:], p[:], 1.0)
            nc.vector.tensor_tensor(acc[:], acc[:], t[:], Alu.mult)
            nc.vector.tensor_tensor(p[:], p[:], p[:], Alu.mult)
        nc.vector.tensor_scalar_add(t[:], p[:], 1.0)
        nc.vector.tensor_tensor(acc[:], acc[:], t[:], Alu.mult)
        a128 = constp.tile([HG, C], f32, tag=f"a128{par}")
        nc.vector.tensor_tensor(a128[:], p[:], p[:], Alu.mult)
        # s0c = 1 + a0*geo  (fp16 copy for the gpsimd col0 scale)
        s0cf = constp.tile([HG, C], f32, tag=f"s0cf{par}")
        nc.vector.tensor_tensor(t[:], a0[:], acc[:], Alu.mult)
        nc.vector.tensor_scalar_add(s0cf[:], t[:], 1.0)
        s0c = constp.tile([HG, C], f16, tag=f"s0c{par}")
        nc.vector.tensor_copy(s0c[:], s0cf[:])
        # qc = mask(a0>=T) * a0^128 * a0
        mask = constp.tile([HG, C], f32, tag=f"mask{par}")
        nc.vector.tensor_scalar(mask[:], a0[:], THRESH, None, Alu.is_ge)
        qcf = constp.tile([HG, C], f32, tag=f"qcf{par}")
        nc.vector.tensor_tensor(qcf[:], mask[:], a128[:], Alu.mult)
        nc.vector.tensor_tensor(qcf[:], qcf[:], a0[:], Alu.mult)
        qc = constp.tile([HG, C], f16, tag=f"qc{par}")
        nc.vector.tensor_copy(qc[:], qcf[:])
        # rec = 1/max(a0, T), broadcast over the K correction columns
        rec = constp.tile([HG, C], f32, tag=f"rec{par}")
        nc.vector.tensor_scalar_max(t[:], a0[:], THRESH)
        nc.vector.reciprocal(rec[:], t[:])
        recb = constp.tile([HG, C * K], f32, tag=f"recb{par}")
        nc.vector.tensor_copy(
            recb[:].rearrange("p (c k) -> p c k", c=C),
            rec[:].unsqueeze(-1).to_broadcast([HG, C, K]))
        coefs[par] = (s0c, qc, recb)

    # persistent correction tiles: qd col0 / qz cols stay zero across iters
    qd = constp.tile([HG, B * (K + 1)], f32, tag="qd")
    qz = constp.tile([HG, B * (K + 1)], f16, tag="qz")
    nc.vector.memset(qd[:], 0.0)
    nc.vector.memset(qz[:], 0.0)
    qdr = qd[:].rearrange("p (c w) -> p c w", c=B)
    qzr = qz[:].rearrange("p (c w) -> p c w", c=B)

    # ---- main loop -----------------------------------------------------
    for blk in range(NBLK):
        c0 = blk * B
        gb = ginp.tile([HG, B * 2 * WPAD], f16, tag="gb")
        gbr = gb[:].rearrange("p (c t w) -> p c t w", c=B, t=2)
        nc.sync.dma_start(gbr, G12d[c0:c0 + B].transpose([1, 0, 2, 3]))

        xb2 = xinp.tile([HG, B * 2 * W], f16, tag="xb2")
        nc.sync.dma_start(
            xb2[:].rearrange("p (c q w) -> p c q w", c=B, q=2),
            Xd[c0:c0 + B].rearrange("c (h q) w -> h c q w", q=2))

        for par in ("e", "o"):
            s0c, qc, recb = coefs[par]
            u3 = weights[par + "3"]
            u1 = weights[par + "1"]
            pstart = 0 if par == "e" else 1

            # PE: H+W upsample straight into PSUM. Layout: [g1u | g2u],
            # each [128, (B,256)] fp32; one matmul pair per 2KB bank.
            # All center-tap (u3) matmuls first, then all shift (u1) ones,
            # to minimize weight switching.
            ps = psp.tile([HG, 2 * B * W], f32, tag="ps")

            def bank_dst(t, cp):
                return ps[:][:, (t * B + cp * 2) * W:(t * B + cp * 2 + 2) * W]

            for t in range(2):
                for cp in range(B // 2):
                    center = _rep_ap(gbr[:, cp * 2, t, 1:2],
                                     [[2 * WPAD, 2], [1, HG], [0, 2]])
                    nc.tensor.matmul(bank_dst(t, cp), u3[:], center,
                                     start=True, stop=False)
            for t in range(2):
                for cp in range(B // 2):
                    shift = _rep_ap(gbr[:, cp * 2, t, 0:1],
                                    [[2 * WPAD, 2], [1, HG], [2, 2]])
                    nc.tensor.matmul(bank_dst(t, cp), u1[:], shift,
                                     start=False, stop=True)
            g1u_ps = ps[:][:, 0:B * W]
            g2u_ps = ps[:][:, B * W:2 * B * W]
            g2u_r = g2u_ps.rearrange("p (c w) -> p c w", c=B)
            # channel-seam reset for the scan carry
            nc.vector.memset(g2u_r[:, :, 0:1], 0.0)

            # ScalarE: PSUM->SBUF copies (g1u cast fp16; g2u correction cols)
            g1u = g1sp.tile([HG, B * W], f16, tag="g1u")
            nc.scalar.copy(g1u[:], g1u_ps)
            g2k = g2kp.tile([HG, B * K], f32, tag="g2k")
            nc.scalar.copy(g2k[:].rearrange("p (c k) -> p c k", c=B),
                           g2u_r[:, :, 1:K + 1])

            # d = g1u * x  (fp16 -> 2x DVE mode)
            xb = xb2[:].rearrange("p (c q w) -> p c q w", c=B, q=2)[:, :, pstart]
            d = datp.tile([HG, B * W], f16, tag="d")
            nc.vector.tensor_tensor(
                d[:].rearrange("p (c w) -> p c w", c=B), g1u[:].rearrange(
                    "p (c w) -> p c w", c=B), xb, Alu.mult)
            dr = d[:].rearrange("p (c w) -> p c w", c=B)

            # correction chain (gpsimd + one small DVE scan)
            # spacer: qz[c,0] = b0*qc  (b0 = d[c,0] pre-scale)
            nc.gpsimd.tensor_tensor(
                qzr[:, :, 0:1], dr[:, :, 0:1],
                qc[:, c0:c0 + B].unsqueeze(-1), Alu.mult)
            # d[c,0] *= s0c (in place, after the spacer read)
            nc.gpsimd.tensor_tensor(
                dr[:, :, 0:1], dr[:, :, 0:1],
                s0c[:, c0:c0 + B].unsqueeze(-1), Alu.mult)
            # qd[c,1:] = g2u[c,1:K+1] * rec
            nc.gpsimd.tensor_tensor(
                qdr[:, :, 1:K + 1],
                g2k[:].rearrange("p (c k) -> p c k", c=B),
                recb[:].rearrange("p (c k) -> p c k", c=C)[:, c0:c0 + B],
                Alu.mult)
            # q[x] via scan; subtract from d
            qo = qop.tile([HG, B * (K + 1)], f16, tag="qo")
            nc.vector.tensor_tensor_scan(
                qo[:], qd[:], qz[:], 0.0, Alu.mult, Alu.add)
            qor = qo[:].rearrange("p (c w) -> p c w", c=B)
            nc.gpsimd.tensor_tensor(
                dr[:, :, 1:K + 1], dr[:, :, 1:K + 1], qor[:, :, 1:K + 1],
                Alu.subtract)

            # main scan: s[x] = g2u[x]*s[x-1] + d[x]
            ot = outp.tile([HG, B * W], f16, tag="ot")
            nc.vector.tensor_tensor_scan(
                ot[:], g2u_ps, d[:], 0.0, Alu.mult, Alu.add)
            nc.sync.dma_start(
                Od[c0:c0 + B, pstart:H:2, :].transpose([1, 0, 2]),
                ot[:].rearrange("p (c w) -> p c w", c=B))


def build():
    nc = bacc.Bacc("TRN2", target_bir_lowering=False, debug=False,
                   num_devices=NCORES)
    Xd = nc.dram_tensor("X", [C, H, W], f16, kind="ExternalInput")
    G12d = nc.dram_tensor("G12", [C, HG, 2, WPAD], f16, kind="ExternalInput")
    Ud = {n: nc.dram_tensor(n.upper(), [HG, HG], f16, kind="ExternalInput")
          for n in ("e3", "e1", "o3", "o1")}
    Od = nc.dram_tensor("O", [C, H, W], f16, kind="ExternalOutput")

    with tile.TileContext(nc) as tc:
        with (
            tc.tile_pool(name="const", bufs=1) as constp,
            tc.tile_pool(name="gin", bufs=3) as ginp,
            tc.tile_pool(name="ps", bufs=2, space="PSUM") as psp,
            tc.tile_pool(name="xin", bufs=3) as xinp,
            tc.tile_pool(name="g1s", bufs=3) as g1sp,
            tc.tile_pool(name="dat", bufs=3) as datp,
            tc.tile_pool(name="g2k", bufs=3) as g2kp,
            tc.tile_pool(name="qo", bufs=3) as qop,
            tc.tile_pool(name="out", bufs=3) as outp,
        ):
            weights = {}
            for n in ("e3", "e1", "o3", "o1"):
                w = constp.tile([HG, HG], f16, tag=f"u{n}")
                nc.sync.dma_start(w[:], Ud[n][:])
                weights[n] = w
            pools = (ginp, psp, xinp, g1sp, datp, g2kp, qop, outp, constp)
            _emit(nc, pools, weights, (Xd, G12d, Od))

    nc.compile()
    return nc


_NC = None


def kernel(X, G1, G2, G3=None, **_):
    global _NC
    if _NC is None:
        _NC = build()
    ue, uo = _upsample_mats()
    wmats = {"E3": (3.0 * ue).astype(np.float16),
             "E1": ue.astype(np.float16),
             "O3": (3.0 * uo).astype(np.float16),
             "O1": uo.astype(np.float16)}

    def pad(G):
        return np.concatenate([G[..., :1], G, G[..., -1:]], axis=-1)

    Xh = np.ascontiguousarray(X).astype(np.float16)
    G12h = np.stack([pad(np.asarray(G1)), pad(np.asarray(G2))],
                    axis=3).astype(np.float16)

    in_maps = [
        {"X": Xh[k], "G12": np.ascontiguousarray(G12h[k]), **wmats}
        for k in range(NCORES)
    ]
    res = run_bass_kernel_spmd(_NC, in_maps, list(range(NCORES)))
    kernel.last_result = res
    out = np.stack([res.results[k]["O"] for k in range(NCORES)])
    return out.astype(np.float32)


# revision 12
# speedup vs baseline: 1.6040x; 1.0195x over previous
"""GateRecurrent2dnoind (horizontal, forward) Trainium2 kernel, v2.

Semantics (matching the reference):
  G1u, G2u = bilinear 2x upsample (half-pixel) of G1, G2 to (256, 256)
  g1x = G1u * X
  o = g1x; repeat 128x: o = g1x + G2u * shift_right_w(o)   (left edge replicated)

The 128 Jacobi passes collapse into ONE sequential scan along W with an exact
depth-128 window emulation (boundary geometric series + per-column window
corrections propagated by the scan; see _emit for the coefficient math).

v2 design (vs the fp32 baseline):
  - fp16 end-to-end for the big tensors (X, G in DRAM, d, g1u, output).
    Measured rel err 1.5e-3 vs the 2e-2 gate.
  - BOTH upsample directions run on the TensorEngine: for each PSUM bank,
    matmul#1 (weights 3U, moving AP with a stride-0 "repeat" dim) writes the
    0.75-weighted center tap to both W-parity slots; matmul#2 (weights U,
    moving AP with a (j:+1, rep:+2) window over a host-padded G tile)
    accumulates the 0.25-weighted left/right taps.  The full upsampled gate
    lands in PSUM with no vector-engine work.
  - The main scan reads g2u directly from PSUM (fp32 data0 = 2 cyc/elem; a
    16-bit data0 would run at 4 cyc/elem), data1 = d (fp16), out fp16.
  - VectorE does only: the two scans + the d = g1u*x multiply (fp16 2x mode)
    + a tiny PSUM col0 memset.  GpSimd does the small correction ops.
    ScalarE does the PSUM->SBUF cast copies.

Sharding: batch b -> core b (8 batches, 8 cores). Per core: [64, 256, 256].
"""

import numpy as np

import concourse.bacc as bacc
import concourse.mybir as mybir
import concourse.tile as tile
from concourse.ap import AP
from concourse.bass_utils import run_bass_kernel_spmd

f32 = mybir.dt.float32
f16 = mybir.dt.float16
Alu = mybir.AluOpType

NCORES = 8
C = 64          # channels per core
H = 256
W = 256
HG = 128        # G input h/w
WPAD = HG + 2   # G w + replicate pads
B = 4           # channels per block
NBLK = C // B
K = 16          # correction columns
THRESH = 0.75   # a0 mask/clamp for the correction chain


def _upsample_mats():
    """[k=in_row, m=out_row] H-upsample matrices, scaled by 0.25.

    even rows: out[m] = 0.25*in[m-1] + 0.75*in[m]   (m=0 clamps to in[0])
    odd rows:  out[m] = 0.75*in[m] + 0.25*in[m+1]   (m=127 clamps to in[127])
    """
    ue = np.zeros((HG, HG), np.float32)
    uo = np.zeros((HG, HG), np.float32)
    for m in range(HG):
        ue[m, m] += 0.25 * 0.75
        ue[max(m - 1, 0), m] += 0.25 * 0.25
        uo[m, m] += 0.25 * 0.75
        uo[min(m + 1, HG - 1), m] += 0.25 * 0.25
    return ue, uo


def _rep_ap(anchor, dims):
    """Raw AP sharing anchor's tensor/offset/partition dim, custom free dims."""
    return AP(anchor.tensor, anchor.offset, [list(anchor.ap[0])] + dims)


def _emit(nc, pools, weights, dram):
    (ginp, psp, xinp, g1sp, datp, g2kp, qop, outp, constp) = pools
    Xd, G12d, Od = dram

    # ---- per-parity boundary precompute (all channels at once) ---------
    # a0 = g2u[..., 0] = 4 * (U_par @ G2[:, :, 0]); the scan's full-history
    # geometric pile-up at the replicated left edge is corrected exactly via
    #   s[0] *= s0c = 1 + a0 * sum_{m=0}^{127} a0^m
    #   d[x] -= q[x] for x=1..K, q[x] = (b0*qc) * prod_{i=1..x}(g2u[i]*rec)
    #   with qc = mask(a0>=T) * a0^129, rec = 1/max(a0, T).
    g2c0 = constp.tile([HG, C], f16, tag="g2c0")
    nc.sync.dma_start(g2c0[:], G12d[:, :, 1, 1].transpose([1, 0]))
    coefs = {}
    for par in ("e", "o"):
        u1 = weights[par + "1"]
        ps = psp.tile([HG, C], f32, tag="ps")
        nc.tensor.matmul(ps[:], u1[:], g2c0[:], start=True, stop=True)
        a0 = constp.tile([HG, C], f32, tag=f"a0{par}")
        nc.vector.tensor_scalar_mul(a0[:], ps[:], 4.0)
        # geo = sum_{m=0}^{127} a0^m = prod_k (1 + a0^(2^k)), k=0..6
        acc = constp.tile([HG, C], f32, tag=f"acc{par}")
        p = constp.tile([HG, C], f32, tag=f"p{par}")
        t = constp.tile([HG, C], f32, tag=f"t{par}")
        nc.vector.tensor_scalar_add(acc[:], a0[:], 1.0)
        nc.vector.tensor_tensor(p[:], a0[:], a0[:], Alu.mult)
        for _ in range(5):
            nc.vector.tensor_scalar_add(t[:], p[:], 1.0)
            nc.vector.tensor_tensor(acc[:], acc[:], t[:], Alu.mult)
            nc.vector.tensor_tensor(p[:], p[:], p[:], Alu.mult)
        nc.vector.tensor_scalar_add(t[:], p[:], 1.0)
        nc.vector.tensor_tensor(acc[:], acc[:], t[:], Alu.mult)
        a128 = constp.tile([HG, C], f32, tag=f"a128{par}")
        nc.vector.tensor_tensor(a128[:], p[:], p[:], Alu.mult)
        # s0c = 1 + a0*geo  (fp16 copy for the gpsimd col0 scale)
        s0cf = constp.tile([HG, C], f32, tag=f"s0cf{par}")
        nc.vector.tensor_tensor(t[:], a0[:], acc[:], Alu.mult)
        nc.vector.tensor_scalar_add(s0cf[:], t[:], 1.0)
        s0c = constp.tile([HG, C], f16, tag=f"s0c{par}")
        nc.vector.tensor_copy(s0c[:], s0cf[:])
        # qc = mask(a0>=T) * a0^128 * a0
        mask = constp.tile([HG, C], f32, tag=f"mask{par}")
        nc.vector.tensor_scalar(mask[:], a0[:], THRESH, None, Alu.is_ge)
        qcf = constp.tile([HG, C], f32, tag=f"qcf{par}")
        nc.vector.tensor_tensor(qcf[:], mask[:], a128[:], Alu.mult)
        nc.vector.tensor_tensor(qcf[:], qcf[:], a0[:], Alu.mult)
        qc = constp.tile([HG, C], f16, tag=f"qc{par}")
        nc.vector.tensor_copy(qc[:], qcf[:])
        # rec = 1/max(a0, T), broadcast over the K correction columns
        rec = constp.tile([HG, C], f32, tag=f"rec{par}")
        nc.vector.tensor_scalar_max(t[:], a0[:], THRESH)
        nc.vector.reciprocal(rec[:], t[:])
        recb = constp.tile([HG, C * K], f32, tag=f"recb{par}")
        nc.vector.tensor_copy(
            recb[:].rearrange("p (c k) -> p c k", c=C),
            rec[:].unsqueeze(-1).to_broadcast([HG, C, K]))
        coefs[par] = (s0c, qc, recb)

    # persistent correction tiles (per parity, to decouple consecutive
    # iterations): qd col0 / qz cols 1.. stay zero across iters
    qdt, qzt = {}, {}
    for par in ("e", "o"):
        qd = constp.tile([HG, B * (K + 1)], f32, tag=f"qd{par}")
        qz = constp.tile([HG, B * (K + 1)], f16, tag=f"qz{par}")
        nc.vector.memset(qd[:], 0.0)
        nc.vector.memset(qz[:], 0.0)
        qdt[par], qzt[par] = qd, qz

    # ---- main loop -----------------------------------------------------
    for blk in range(NBLK):
        c0 = blk * B
        gb = ginp.tile([HG, B * 2 * WPAD], f16, tag="gb")
        gbr = gb[:].rearrange("p (c t w) -> p c t w", c=B, t=2)
        nc.sync.dma_start(gbr, G12d[c0:c0 + B].transpose([1, 0, 2, 3]))

        xb2 = xinp.tile([HG, B * 2 * W], f16, tag="xb2")
        nc.sync.dma_start(
            xb2[:].rearrange("p (c q w) -> p c q w", c=B, q=2),
            Xd[c0:c0 + B].rearrange("c (h q) w -> h c q w", q=2))

        for par in ("e", "o"):
            s0c, qc, recb = coefs[par]
            qd, qz = qdt[par], qzt[par]
            qdr = qd[:].rearrange("p (c w) -> p c w", c=B)
            qzr = qz[:].rearrange("p (c w) -> p c w", c=B)
            u3 = weights[par + "3"]
            u1 = weights[par + "1"]
            pstart = 0 if par == "e" else 1

            # PE: H+W upsample straight into PSUM. Layout: [g1u | g2u],
            # each [128, (B,256)] fp32; one matmul pair per 2KB bank.
            # All center-tap (u3) matmuls first, then all shift (u1) ones,
            # to minimize weight switching.
            ps = psp.tile([HG, 2 * B * W], f32, tag="ps")

            def bank_dst(t, cp):
                return ps[:][:, (t * B + cp * 2) * W:(t * B + cp * 2 + 2) * W]

            for t in range(2):
                for cp in range(B // 2):
                    center = _rep_ap(gbr[:, cp * 2, t, 1:2],
                                     [[2 * WPAD, 2], [1, HG], [0, 2]])
                    nc.tensor.matmul(bank_dst(t, cp), u3[:], center,
                                     start=True, stop=False)
                for cp in range(B // 2):
                    shift = _rep_ap(gbr[:, cp * 2, t, 0:1],
                                    [[2 * WPAD, 2], [1, HG], [2, 2]])
                    nc.tensor.matmul(bank_dst(t, cp), u1[:], shift,
                                     start=False, stop=True)
            g1u_ps = ps[:][:, 0:B * W]
            g2u_ps = ps[:][:, B * W:2 * B * W]
            g2u_r = g2u_ps.rearrange("p (c w) -> p c w", c=B)
            # channel-seam reset for the scan carry
            nc.vector.memset(g2u_r[:, :, 0:1], 0.0)

            # ScalarE: PSUM->SBUF copies (g1u cast fp16; g2u correction cols)
            g1u = g1sp.tile([HG, B * W], f16, tag="g1u")
            nc.scalar.copy(g1u[:], g1u_ps)
            g2k = g2kp.tile([HG, B * K], f32, tag="g2k")
            nc.scalar.copy(g2k[:].rearrange("p (c k) -> p c k", c=B),
                           g2u_r[:, :, 1:K + 1])

            # d = g1u * x  (fp16 -> 2x DVE mode)
            xb = xb2[:].rearrange("p (c q w) -> p c q w", c=B, q=2)[:, :, pstart]
            d = datp.tile([HG, B * W], f16, tag="d")
            nc.vector.tensor_tensor(
                d[:].rearrange("p (c w) -> p c w", c=B), g1u[:].rearrange(
                    "p (c w) -> p c w", c=B), xb, Alu.mult)
            dr = d[:].rearrange("p (c w) -> p c w", c=B)

            # correction chain (gpsimd + one small DVE scan)
            # spacer: qz[c,0] = b0*qc  (b0 = d[c,0] pre-scale)
            nc.gpsimd.tensor_tensor(
                qzr[:, :, 0:1], dr[:, :, 0:1],
                qc[:, c0:c0 + B].unsqueeze(-1), Alu.mult)
            # d[c,0] *= s0c (in place, after the spacer read)
            nc.gpsimd.tensor_tensor(
                dr[:, :, 0:1], dr[:, :, 0:1],
                s0c[:, c0:c0 + B].unsqueeze(-1), Alu.mult)
            # qd[c,1:] = g2u[c,1:K+1] * rec
            nc.gpsimd.tensor_tensor(
                qdr[:, :, 1:K + 1],
                g2k[:].rearrange("p (c k) -> p c k", c=B),
                recb[:].rearrange("p (c k) -> p c k", c=C)[:, c0:c0 + B],
                Alu.mult)
            # q[x] via scan; subtract from d
            qo = qop.tile([HG, B * (K + 1)], f16, tag="qo")
            nc.vector.tensor_tensor_scan(
                qo[:], qd[:], qz[:], 0.0, Alu.mult, Alu.add)
            qor = qo[:].rearrange("p (c w) -> p c w", c=B)
            nc.gpsimd.tensor_tensor(
                dr[:, :, 1:K + 1], dr[:, :, 1:K + 1], qor[:, :, 1:K + 1],
                Alu.subtract)

            # main scan: s[x] = g2u[x]*s[x-1] + d[x]
            ot = outp.tile([HG, B * W], f16, tag="ot")
            nc.vector.tensor_tensor_scan(
                ot[:], g2u_ps, d[:], 0.0, Alu.mult, Alu.add)
            nc.sync.dma_start(
                Od[c0:c0 + B, pstart:H:2, :].transpose([1, 0, 2]),
                ot[:].rearrange("p (c w) -> p c w", c=B))


def build():
    nc = bacc.Bacc("TRN2", target_bir_lowering=False, debug=False,
                   num_devices=NCORES)
    Xd = nc.dram_tensor("X", [C, H, W], f16, kind="ExternalInput")
    G12d = nc.dram_tensor("G12", [C, HG, 2, WPAD], f16, kind="ExternalInput")
    Ud = {n: nc.dram_tensor(n.upper(), [HG, HG], f16, kind="ExternalInput")
          for n in ("e3", "e1", "o3", "o1")}
    Od = nc.dram_tensor("O", [C, H, W], f16, kind="ExternalOutput")

    with tile.TileContext(nc) as tc:
        with (
            tc.tile_pool(name="const", bufs=1) as constp,
            tc.tile_pool(name="gin", bufs=3) as ginp,
            tc.tile_pool(name="ps", bufs=2, space="PSUM") as psp,
            tc.tile_pool(name="xin", bufs=3) as xinp,
            tc.tile_pool(name="g1s", bufs=4) as g1sp,
            tc.tile_pool(name="dat", bufs=4) as datp,
            tc.tile_pool(name="g2k", bufs=4) as g2kp,
            tc.tile_pool(name="qo", bufs=3) as qop,
            tc.tile_pool(name="out", bufs=4) as outp,
        ):
            weights = {}
            for n in ("e3", "e1", "o3", "o1"):
                w = constp.tile([HG, HG], f16, tag=f"u{n}")
                nc.sync.dma_start(w[:], Ud[n][:])
                weights[n] = w
            pools = (ginp, psp, xinp, g1sp, datp, g2kp, qop, outp, constp)
            _emit(nc, pools, weights, (Xd, G12d, Od))

    nc.compile()
    return nc


_NC = None


def kernel(X, G1, G2, G3=None, **_):
    global _NC
    if _NC is None:
        _NC = build()
    ue, uo = _upsample_mats()
    wmats = {"E3": (3.0 * ue).astype(np.float16),
             "E1": ue.astype(np.float16),
             "O3": (3.0 * uo).astype(np.float16),
             "O1": uo.astype(np.float16)}

    def pad(G):
        return np.concatenate([G[..., :1], G, G[..., -1:]], axis=-1)

    Xh = np.ascontiguousarray(X).astype(np.float16)
    G12h = np.stack([pad(np.asarray(G1)), pad(np.asarray(G2))],
                    axis=3).astype(np.float16)

    in_maps = [
        {"X": Xh[k], "G12": np.ascontiguousarray(G12h[k]), **wmats}
        for k in range(NCORES)
    ]
    res = run_bass_kernel_spmd(_NC, in_maps, list(range(NCORES)))
    kernel.last_result = res
    out = np.stack([res.results[k]["O"] for k in range(NCORES)])
    return out.astype(np.float32)


# revision 15
# speedup vs baseline: 1.6158x; 1.0074x over previous
"""GateRecurrent2dnoind (horizontal, forward) Trainium2 kernel, v2.

Semantics (matching the reference):
  G1u, G2u = bilinear 2x upsample (half-pixel) of G1, G2 to (256, 256)
  g1x = G1u * X
  o = g1x; repeat 128x: o = g1x + G2u * shift_right_w(o)   (left edge replicated)

The 128 Jacobi passes collapse into ONE sequential scan along W with an exact
depth-128 window emulation (boundary geometric series + per-column window
corrections propagated by the scan; see _emit for the coefficient math).

v2 design (vs the fp32 baseline):
  - fp16 end-to-end for the big tensors (X, G in DRAM, d, g1u, output).
    Measured rel err 1.5e-3 vs the 2e-2 gate.
  - BOTH upsample directions run on the TensorEngine: for each PSUM bank,
    matmul#1 (weights 3U, moving AP with a stride-0 "repeat" dim) writes the
    0.75-weighted center tap to both W-parity slots; matmul#2 (weights U,
    moving AP with a (j:+1, rep:+2) window over a host-padded G tile)
    accumulates the 0.25-weighted left/right taps.  The full upsampled gate
    lands in PSUM with no vector-engine work.
  - The main scan reads g2u directly from PSUM (fp32 data0 = 2 cyc/elem; a
    16-bit data0 would run at 4 cyc/elem), data1 = d (fp16), out fp16.
  - VectorE does only: the two scans + the d = g1u*x multiply (fp16 2x mode)
    + a tiny PSUM col0 memset.  GpSimd does the small correction ops.
    ScalarE does the PSUM->SBUF cast copies.

Sharding: batch b -> core b (8 batches, 8 cores). Per core: [64, 256, 256].
"""

import numpy as np

import concourse.bacc as bacc
import concourse.mybir as mybir
import concourse.tile as tile
from concourse.ap import AP
from concourse.bass_utils import run_bass_kernel_spmd

f32 = mybir.dt.float32
f16 = mybir.dt.float16
Alu = mybir.AluOpType

NCORES = 8
C = 64          # channels per core
H = 256
W = 256
HG = 128        # G input h/w
WPAD = HG + 2   # G w + replicate pads
B = 4           # channels per block
NBLK = C // B
K = 16          # correction columns
THRESH = 0.75   # a0 mask/clamp for the correction chain


def _upsample_mats():
    """[k=in_row, m=out_row] H-upsample matrices, scaled by 0.25.

    even rows: out[m] = 0.25*in[m-1] + 0.75*in[m]   (m=0 clamps to in[0])
    odd rows:  out[m] = 0.75*in[m] + 0.25*in[m+1]   (m=127 clamps to in[127])
    """
    ue = np.zeros((HG, HG), np.float32)
    uo = np.zeros((HG, HG), np.float32)
    for m in range(HG):
        ue[m, m] += 0.25 * 0.75
        ue[max(m - 1, 0), m] += 0.25 * 0.25
        uo[m, m] += 0.25 * 0.75
        uo[min(m + 1, HG - 1), m] += 0.25 * 0.25
    return ue, uo


def _rep_ap(anchor, dims):
    """Raw AP sharing anchor's tensor/offset/partition dim, custom free dims."""
    return AP(anchor.tensor, anchor.offset, [list(anchor.ap[0])] + dims)


def _emit(nc, pools, weights, dram):
    (ginp, ps1p, ps2p, xinp, g1sp, datp, g2kp, qop, outp, constp) = pools
    Xd, G12d, Od = dram

    # ---- per-parity boundary precompute (all channels at once) ---------
    # a0 = g2u[..., 0] = 4 * (U_par @ G2[:, :, 0]); the scan's full-history
    # geometric pile-up at the replicated left edge is corrected exactly via
    #   s[0] *= s0c = 1 + a0 * sum_{m=0}^{127} a0^m
    #   d[x] -= q[x] for x=1..K, q[x] = (b0*qc) * prod_{i=1..x}(g2u[i]*rec)
    #   with qc = mask(a0>=T) * a0^129, rec = 1/max(a0, T).
    g2c0 = constp.tile([HG, C], f16, tag="g2c0")
    nc.sync.dma_start(g2c0[:], G12d[:, :, 1, 1].transpose([1, 0]))
    coefs = {}
    for par in ("e", "o"):
        u1 = weights[par + "1"]
        ps = ps2p.tile([HG, C], f32, tag="ps2")
        nc.tensor.matmul(ps[:], u1[:], g2c0[:], start=True, stop=True)
        a0 = constp.tile([HG, C], f32, tag=f"a0{par}")
        nc.vector.tensor_scalar_mul(a0[:], ps[:], 4.0)
        # geo = sum_{m=0}^{127} a0^m = prod_k (1 + a0^(2^k)), k=0..6
        acc = constp.tile([HG, C], f32, tag=f"acc{par}")
        p = constp.tile([HG, C], f32, tag=f"p{par}")
        t = constp.tile([HG, C], f32, tag=f"t{par}")
        nc.vector.tensor_scalar_add(acc[:], a0[:], 1.0)
        nc.vector.tensor_tensor(p[:], a0[:], a0[:], Alu.mult)
        for _ in range(5):
            nc.vector.tensor_scalar_add(t[:], p[:], 1.0)
            nc.vector.tensor_tensor(acc[:], acc[:], t[:], Alu.mult)
            nc.vector.tensor_tensor(p[:], p[:], p[:], Alu.mult)
        nc.vector.tensor_scalar_add(t[:], p[:], 1.0)
        nc.vector.tensor_tensor(acc[:], acc[:], t[:], Alu.mult)
        a128 = constp.tile([HG, C], f32, tag=f"a128{par}")
        nc.vector.tensor_tensor(a128[:], p[:], p[:], Alu.mult)
        # s0c = 1 + a0*geo  (fp16 copy for the gpsimd col0 scale)
        s0cf = constp.tile([HG, C], f32, tag=f"s0cf{par}")
        nc.vector.tensor_tensor(t[:], a0[:], acc[:], Alu.mult)
        nc.vector.tensor_scalar_add(s0cf[:], t[:], 1.0)
        s0c = constp.tile([HG, C], f16, tag=f"s0c{par}")
        nc.vector.tensor_copy(s0c[:], s0cf[:])
        # qc = mask(a0>=T) * a0^128 * a0
        mask = constp.tile([HG, C], f32, tag=f"mask{par}")
        nc.vector.tensor_scalar(mask[:], a0[:], THRESH, None, Alu.is_ge)
        qcf = constp.tile([HG, C], f32, tag=f"qcf{par}")
        nc.vector.tensor_tensor(qcf[:], mask[:], a128[:], Alu.mult)
        nc.vector.tensor_tensor(qcf[:], qcf[:], a0[:], Alu.mult)
        qc = constp.tile([HG, C], f16, tag=f"qc{par}")
        nc.vector.tensor_copy(qc[:], qcf[:])
        # rec = 1/max(a0, T), broadcast over the K correction columns
        rec = constp.tile([HG, C], f32, tag=f"rec{par}")
        nc.vector.tensor_scalar_max(t[:], a0[:], THRESH)
        nc.vector.reciprocal(rec[:], t[:])
        recb = constp.tile([HG, C * K], f32, tag=f"recb{par}")
        nc.vector.tensor_copy(
            recb[:].rearrange("p (c k) -> p c k", c=C),
            rec[:].unsqueeze(-1).to_broadcast([HG, C, K]))
        coefs[par] = (s0c, qc, recb)

    # persistent correction tiles (per parity, to decouple consecutive
    # iterations): qd col0 / qz cols 1.. stay zero across iters
    qdt, qzt = {}, {}
    for par in ("e", "o"):
        qd = constp.tile([HG, B * (K + 1)], f32, tag=f"qd{par}")
        qz = constp.tile([HG, B * (K + 1)], f16, tag=f"qz{par}")
        nc.vector.memset(qd[:], 0.0)
        nc.vector.memset(qz[:], 0.0)
        qdt[par], qzt[par] = qd, qz

    # ---- main loop -----------------------------------------------------
    for blk in range(NBLK):
        c0 = blk * B
        gb = ginp.tile([HG, B * 2 * WPAD], f16, tag="gb")
        gbr = gb[:].rearrange("p (c t w) -> p c t w", c=B, t=2)
        nc.sync.dma_start(gbr, G12d[c0:c0 + B].transpose([1, 0, 2, 3]))

        xb2 = xinp.tile([HG, B * 2 * W], f16, tag="xb2")
        nc.sync.dma_start(
            xb2[:].rearrange("p (c q w) -> p c q w", c=B, q=2),
            Xd[c0:c0 + B].rearrange("c (h q) w -> h c q w", q=2))

        for par in ("e", "o"):
            s0c, qc, recb = coefs[par]
            qd, qz = qdt[par], qzt[par]
            qdr = qd[:].rearrange("p (c w) -> p c w", c=B)
            qzr = qz[:].rearrange("p (c w) -> p c w", c=B)
            u3 = weights[par + "3"]
            u1 = weights[par + "1"]
            pstart = 0 if par == "e" else 1

            # PE: H+W upsample straight into PSUM, g1u and g2u in separate
            # pools so their consumers don't serialize on one tile.  One
            # matmul pair (center tap u3, shifted taps u1) per 2KB bank.
            ps1 = ps1p.tile([HG, B * W], f32, tag="ps1")
            ps2 = ps2p.tile([HG, B * W], f32, tag="ps2")

            for t, ps in ((0, ps1), (1, ps2)):
                for cp in range(B // 2):
                    dst = ps[:][:, cp * 2 * W:(cp * 2 + 2) * W]
                    center = _rep_ap(gbr[:, cp * 2, t, 1:2],
                                     [[2 * WPAD, 2], [1, HG], [0, 2]])
                    nc.tensor.matmul(dst, u3[:], center,
                                     start=True, stop=False)
                for cp in range(B // 2):
                    dst = ps[:][:, cp * 2 * W:(cp * 2 + 2) * W]
                    shift = _rep_ap(gbr[:, cp * 2, t, 0:1],
                                    [[2 * WPAD, 2], [1, HG], [2, 2]])
                    nc.tensor.matmul(dst, u1[:], shift,
                                     start=False, stop=True)
            g2u_r = ps2[:].rearrange("p (c w) -> p c w", c=B)
            # channel-seam reset for the scan carry
            nc.vector.memset(g2u_r[:, :, 0:1], 0.0)

            # ScalarE: PSUM->SBUF copies (g1u cast fp16; g2u correction cols)
            g1u = g1sp.tile([HG, B * W], f16, tag="g1u")
            nc.scalar.copy(g1u[:], ps1[:])
            g2k = g2kp.tile([HG, B * K], f32, tag="g2k")
            nc.scalar.copy(g2k[:].rearrange("p (c k) -> p c k", c=B),
                           g2u_r[:, :, 1:K + 1])

            g1ur = g1u[:].rearrange("p (c w) -> p c w", c=B)
            xb = xb2[:].rearrange("p (c q w) -> p c q w", c=B, q=2)[:, :, pstart]
            d = datp.tile([HG, B * W], f16, tag="d")
            dr = d[:].rearrange("p (c w) -> p c w", c=B)

            # gpsimd pre-chain (overlaps the previous main scan):
            # b0 = g1u[.,0]*x[.,0]; spacer qz0 = b0*qc; d[.,0] = b0*s0c;
            # qd[.,1:] = g2u[.,1:K+1]*rec
            b0 = g2kp.tile([HG, B], f16, tag="b0")
            nc.gpsimd.tensor_tensor(
                b0[:].unsqueeze(-1), g1ur[:, :, 0:1], xb[:, :, 0:1], Alu.mult)
            nc.gpsimd.tensor_tensor(
                qzr[:, :, 0:1], b0[:].unsqueeze(-1),
                qc[:, c0:c0 + B].unsqueeze(-1), Alu.mult)
            nc.gpsimd.tensor_tensor(
                dr[:, :, 0:1], b0[:].unsqueeze(-1),
                s0c[:, c0:c0 + B].unsqueeze(-1), Alu.mult)
            nc.gpsimd.tensor_tensor(
                qdr[:, :, 1:K + 1],
                g2k[:].rearrange("p (c k) -> p c k", c=B),
                recb[:].rearrange("p (c k) -> p c k", c=C)[:, c0:c0 + B],
                Alu.mult)

            # DVE: correction scan, then d = g1u*x in two pieces so the
            # subtract (gpsimd) overlaps the big tail multiply
            qo = qop.tile([HG, B * (K + 1)], f16, tag="qo")
            nc.vector.tensor_tensor_scan(
                qo[:], qd[:], qz[:], 0.0, Alu.mult, Alu.add)
            qor = qo[:].rearrange("p (c w) -> p c w", c=B)
            nc.vector.tensor_tensor(
                dr[:, :, 1:K + 2], g1ur[:, :, 1:K + 2], xb[:, :, 1:K + 2],
                Alu.mult)
            nc.gpsimd.tensor_tensor(
                dr[:, :, 1:K + 1], dr[:, :, 1:K + 1], qor[:, :, 1:K + 1],
                Alu.subtract)
            nc.vector.tensor_tensor(
                dr[:, :, K + 2:], g1ur[:, :, K + 2:], xb[:, :, K + 2:],
                Alu.mult)

            # main scan: s[x] = g2u[x]*s[x-1] + d[x]
            ot = outp.tile([HG, B * W], f16, tag="ot")
            nc.vector.tensor_tensor_scan(
                ot[:], ps2[:], d[:], 0.0, Alu.mult, Alu.add)
            nc.sync.dma_start(
                Od[c0:c0 + B, pstart:H:2, :].transpose([1, 0, 2]),
                ot[:].rearrange("p (c w) -> p c w", c=B))


def build():
    nc = bacc.Bacc("TRN2", target_bir_lowering=False, debug=False,
                   num_devices=NCORES)
    Xd = nc.dram_tensor("X", [C, H, W], f16, kind="ExternalInput")
    G12d = nc.dram_tensor("G12", [C, HG, 2, WPAD], f16, kind="ExternalInput")
    Ud = {n: nc.dram_tensor(n.upper(), [HG, HG], f16, kind="ExternalInput")
          for n in ("e3", "e1", "o3", "o1")}
    Od = nc.dram_tensor("O", [C, H, W], f16, kind="ExternalOutput")

    with tile.TileContext(nc) as tc:
        with (
            tc.tile_pool(name="const", bufs=1) as constp,
            tc.tile_pool(name="gin", bufs=3) as ginp,
            tc.tile_pool(name="ps1", bufs=2, space="PSUM") as ps1p,
            tc.tile_pool(name="ps2", bufs=2, space="PSUM") as ps2p,
            tc.tile_pool(name="xin", bufs=3) as xinp,
            tc.tile_pool(name="g1s", bufs=4) as g1sp,
            tc.tile_pool(name="dat", bufs=4) as datp,
            tc.tile_pool(name="g2k", bufs=4) as g2kp,
            tc.tile_pool(name="qo", bufs=3) as qop,
            tc.tile_pool(name="out", bufs=4) as outp,
        ):
            weights = {}
            for n in ("e3", "e1", "o3", "o1"):
                w = constp.tile([HG, HG], f16, tag=f"u{n}")
                nc.sync.dma_start(w[:], Ud[n][:])
                weights[n] = w
            pools = (ginp, ps1p, ps2p, xinp, g1sp, datp, g2kp, qop, outp,
                     constp)
            _emit(nc, pools, weights, (Xd, G12d, Od))

    nc.compile()
    return nc


_NC = None


def kernel(X, G1, G2, G3=None, **_):
    global _NC
    if _NC is None:
        _NC = build()
    ue, uo = _upsample_mats()
    wmats = {"E3": (3.0 * ue).astype(np.float16),
             "E1": ue.astype(np.float16),
             "O3": (3.0 * uo).astype(np.float16),
             "O1": uo.astype(np.float16)}

    def pad(G):
        return np.concatenate([G[..., :1], G, G[..., -1:]], axis=-1)

    Xh = np.ascontiguousarray(X).astype(np.float16)
    G12h = np.stack([pad(np.asarray(G1)), pad(np.asarray(G2))],
                    axis=3).astype(np.float16)

    in_maps = [
        {"X": Xh[k], "G12": np.ascontiguousarray(G12h[k]), **wmats}
        for k in range(NCORES)
    ]
    res = run_bass_kernel_spmd(_NC, in_maps, list(range(NCORES)))
    kernel.last_result = res
    out = np.stack([res.results[k]["O"] for k in range(NCORES)])
    return out.astype(np.float32)


# revision 19
# speedup vs baseline: 2.1920x; 1.3566x over previous
"""GateRecurrent2dnoind (horizontal, forward) Trainium2 kernel, v6.

Semantics (matching the reference):
  G1u, G2u = bilinear 2x upsample (half-pixel) of G1, G2 to (256, 256)
  g1x = G1u * X
  o = g1x; repeat 128x: o = g1x + G2u * shift_right_w(o)   (left edge replicated)

The 128 Jacobi passes collapse into ONE sequential scan along W with an exact
depth-128 window emulation:
  s[x] = d[x] + g2u[x]*s[x-1],  d = g1u*X  with
  d[0] scaled by s0c = 1 + a0*sum_{m=0}^{127} a0^m   (a0 = g2u[.,0]) and
  d[x] -= q[x] for x=1..K, q[x] = (b0*qc)*prod_{i=1..x}(g2u[i]*rec),
  qc = mask(a0>=T)*a0^129, rec = 1/max(a0,T)  (window-excess correction).

Design highlights (measured on HW):
  - fp16 end-to-end for the big tensors (rel err 1.5e-3 vs the 2e-2 gate).
  - Both upsample directions run on the TensorEngine: per 2KB PSUM bank,
    matmul#1 (weights 3U, moving AP with a stride-0 repeat dim) writes the
    center tap to both W-parity slots, matmul#2 (weights U, a (j:+1, rep:+2)
    window over host-padded 130-col G tiles) accumulates the side taps.
  - The main scan reads g2u straight from PSUM (fp32 data0 = 2 cyc/elem;
    16-bit data0 would be 4 cyc/elem), data1 = d fp16, out fp16.
  - The ENTIRE correction chain (b0, spacers, qd, and all 32 correction
    scans) is precomputed per parity from G1/G2/X column slices, so the
    steady-state loop is only: 8 matmuls (PE), g1u cast (ACT), memset +
    2-piece d-multiply + main scan (DVE), col0 copy + subtract (GpSimd).

Sharding: batch b -> core b (8 batches, 8 cores). Per core: [64, 256, 256].
"""

import numpy as np

import concourse.bacc as bacc
import concourse.mybir as mybir
import concourse.tile as tile
from concourse.ap import AP
from concourse.bass_utils import run_bass_kernel_spmd

f32 = mybir.dt.float32
f16 = mybir.dt.float16
Alu = mybir.AluOpType

NCORES = 8
C = 64          # channels per core
H = 256
W = 256
HG = 128        # G input h/w
WPAD = HG + 2   # G w + replicate pads
B = 4           # channels per block
NBLK = C // B
K = 16          # correction columns
KP = K + 1      # correction scan width per channel
THRESH = 0.75   # a0 mask/clamp for the correction chain
NG2C = 11       # padded G2 columns needed for g2u[0..17] (pad + cols 0..9)


def _upsample_mats():
    """[k=in_row, m=out_row] H-upsample matrices, scaled by 0.25.

    even rows: out[m] = 0.25*in[m-1] + 0.75*in[m]   (m=0 clamps to in[0])
    odd rows:  out[m] = 0.75*in[m] + 0.25*in[m+1]   (m=127 clamps to in[127])
    """
    ue = np.zeros((HG, HG), np.float32)
    uo = np.zeros((HG, HG), np.float32)
    for m in range(HG):
        ue[m, m] += 0.25 * 0.75
        ue[max(m - 1, 0), m] += 0.25 * 0.25
        uo[m, m] += 0.25 * 0.75
        uo[min(m + 1, HG - 1), m] += 0.25 * 0.25
    return ue, uo


def _rep_ap(anchor, dims):
    """Raw AP sharing anchor's tensor/offset/partition dim, custom free dims."""
    return AP(anchor.tensor, anchor.offset, [list(anchor.ap[0])] + dims)


def _precompute(nc, ps2p, constp, weights, Auxd):
    """Per-parity boundary coefficients + the full correction tables.

    Aux layout (host-packed, [128, 2+NG2C+2 per channel] fp16 c-major):
      [c,0] = G1[c,:,0]; [c,1] = G2[c,:,0]; [c,2:2+NG2C] = padded G2 head;
      [c,-2] = X[c,0::2,0]; [c,-1] = X[c,1::2,0].
    Returns {par: (d0_all [128,C] f16, qo_all [128,C*KP] f16)}.
    """
    FA = 2 + NG2C + 2
    aux = constp.tile([HG, C * FA], f16, tag="aux")
    nc.sync.dma_start(aux[:], Auxd[:])
    auxr = aux[:].rearrange("p (c f) -> p c f", c=C)
    g1c0 = auxr[:, :, 0]
    g2c0 = auxr[:, :, 1]
    ghr = auxr[:, :, 2:2 + NG2C]

    out = {}
    for par in ("e", "o"):
        u1 = weights[par + "1"]
        xc0 = auxr[:, :, FA - 2 if par == "e" else FA - 1]

        # a0 = 4*(U @ g2c0); a1_0 = 4*(U @ g1c0)  (= g1u[.,0])
        ps = ps2p.tile([HG, C], f32, tag="ps2")
        nc.tensor.matmul(ps[:], u1[:], g2c0, start=True, stop=True)
        a0 = constp.tile([HG, C], f32, tag=f"a0{par}")
        nc.vector.tensor_scalar_mul(a0[:], ps[:], 4.0)
        ps1b = ps2p.tile([HG, C], f32, tag="ps2")
        nc.tensor.matmul(ps1b[:], u1[:], g1c0, start=True, stop=True)
        b0 = constp.tile([HG, C], f32, tag=f"b0{par}")
        nc.vector.tensor_scalar_mul(b0[:], ps1b[:], 4.0)
        xc0f = constp.tile([HG, C], f32, tag=f"xc0f{par}")
        nc.vector.tensor_copy(xc0f[:], xc0)
        nc.vector.tensor_tensor(b0[:], b0[:], xc0f[:], Alu.mult)

        # geo = sum_{m=0}^{127} a0^m = prod_k (1 + a0^(2^k)), k=0..6
        acc = constp.tile([HG, C], f32, tag=f"acc{par}")
        p = constp.tile([HG, C], f32, tag=f"p{par}")
        t = constp.tile([HG, C], f32, tag=f"t{par}")
        nc.vector.tensor_scalar_add(acc[:], a0[:], 1.0)
        nc.vector.tensor_tensor(p[:], a0[:], a0[:], Alu.mult)
        for _ in range(5):
            nc.vector.tensor_scalar_add(t[:], p[:], 1.0)
            nc.vector.tensor_tensor(acc[:], acc[:], t[:], Alu.mult)
            nc.vector.tensor_tensor(p[:], p[:], p[:], Alu.mult)
        nc.vector.tensor_scalar_add(t[:], p[:], 1.0)
        nc.vector.tensor_tensor(acc[:], acc[:], t[:], Alu.mult)
        a128 = constp.tile([HG, C], f32, tag=f"a128{par}")
        nc.vector.tensor_tensor(a128[:], p[:], p[:], Alu.mult)
        # d0_all = b0 * (1 + a0*geo)
        s0cf = constp.tile([HG, C], f32, tag=f"s0cf{par}")
        nc.vector.tensor_tensor(t[:], a0[:], acc[:], Alu.mult)
        nc.vector.tensor_scalar_add(s0cf[:], t[:], 1.0)
        d0_all = constp.tile([HG, C], f16, tag=f"d0a{par}")
        nc.vector.tensor_tensor(s0cf[:], s0cf[:], b0[:], Alu.mult)
        nc.vector.tensor_copy(d0_all[:], s0cf[:])
        # spacer = b0 * mask(a0>=T) * a0^129
        mask = constp.tile([HG, C], f32, tag=f"mask{par}")
        nc.vector.tensor_scalar(mask[:], a0[:], THRESH, None, Alu.is_ge)
        qcf = constp.tile([HG, C], f32, tag=f"qcf{par}")
        nc.vector.tensor_tensor(qcf[:], mask[:], a128[:], Alu.mult)
        nc.vector.tensor_tensor(qcf[:], qcf[:], a0[:], Alu.mult)
        nc.vector.tensor_tensor(qcf[:], qcf[:], b0[:], Alu.mult)
        # rec = 1/max(a0, T)
        rec = constp.tile([HG, C], f32, tag=f"rec{par}")
        nc.vector.tensor_scalar_max(t[:], a0[:], THRESH)
        nc.vector.reciprocal(rec[:], t[:])

        # g2u[w] for w=0..17, all channels: H-up matmul on the padded head
        # columns (two channel-halves to fit PSUM banks), then the W-blend
        # as one scalar_tensor_tensor per half.
        g2k18 = constp.tile([HG, C * 18], f32, tag=f"g2k18{par}")
        g2k18r = g2k18[:].rearrange("p (c w) -> p c w", c=C)
        for half in range(2):
            ch0 = half * (C // 2)
            psh = ps2p.tile([HG, (C // 2) * NG2C], f32, tag="ps2")
            nc.tensor.matmul(
                psh[:], u1[:],
                ghr[:, ch0:ch0 + C // 2], start=True, stop=True)

            c2s = constp.tile([HG, (C // 2) * NG2C], f32, tag=f"c2s{par}")
            nc.scalar.copy(c2s[:], psh[:])
            c2r = c2s[:].rearrange("p (c w) -> p c w", c=C // 2)
            # out[c, 2j+r] = 3*c2[j+1] + c2[j + 2r],  j=0..8  (3D APs only:
            # one scalar_tensor_tensor per W-parity)
            dst = g2k18r[:, ch0:ch0 + C // 2]
            nc.vector.scalar_tensor_tensor(
                dst[:, :, 0:17:2], c2r[:, :, 1:10], 3.0, c2r[:, :, 0:9],
                Alu.mult, Alu.add)
            nc.vector.scalar_tensor_tensor(
                dst[:, :, 1:18:2], c2r[:, :, 1:10], 3.0, c2r[:, :, 2:11],
                Alu.mult, Alu.add)

        # qd_all[c, 0] = 0; qd_all[c, 1..K] = g2u[c, w]*rec
        qd_all = constp.tile([HG, C * KP], f32, tag=f"qda{par}")
        nc.vector.memset(qd_all[:], 0.0)
        qdr = qd_all[:].rearrange("p (c w) -> p c w", c=C)
        nc.vector.tensor_tensor(
            qdr[:, :, 1:KP], g2k18r[:, :, 1:K + 1],
            rec[:].unsqueeze(-1).to_broadcast([HG, C, K]), Alu.mult)
        # qz_all: zeros except col0 = spacer
        qz_all = constp.tile([HG, C * KP], f16, tag=f"qza{par}")
        nc.vector.memset(qz_all[:], 0.0)
        qzr = qz_all[:].rearrange("p (c w) -> p c w", c=C)
        nc.vector.tensor_copy(qzr[:, :, 0:1], qcf[:].unsqueeze(-1))
        # all 32 correction scans in one go
        qo_all = constp.tile([HG, C * KP], f16, tag=f"qoa{par}")
        nc.vector.tensor_tensor_scan(
            qo_all[:], qd_all[:], qz_all[:], 0.0, Alu.mult, Alu.add)
        out[par] = (d0_all, qo_all)
    return out


def _emit(nc, pools, weights, dram):
    (ginp, ps1p, ps2p, xinp, g1sp, datp, outp, constp) = pools
    Xd, G12d, Auxd, Od = dram

    corr = _precompute(nc, ps2p, constp, weights, Auxd)

    for blk in range(NBLK):
        c0 = blk * B
        gb = ginp.tile([HG, B * 2 * WPAD], f16, tag="gb")
        gbr = gb[:].rearrange("p (c t w) -> p c t w", c=B, t=2)
        nc.scalar.dma_start(gbr, G12d[c0:c0 + B].transpose([1, 0, 2, 3]))

        xb2 = xinp.tile([HG, B * 2 * W], f16, tag="xb2")
        nc.scalar.dma_start(
            xb2[:].rearrange("p (c q w) -> p c q w", c=B, q=2),
            Xd[c0:c0 + B].rearrange("c (h q) w -> h c q w", q=2))

        for par in ("e", "o"):
            d0_all, qo_all = corr[par]
            u3 = weights[par + "3"]
            u1 = weights[par + "1"]
            pstart = 0 if par == "e" else 1

            # PE: H+W upsample straight into PSUM (g1u / g2u in separate
            # pools); one matmul pair per 2KB bank.
            ps1 = ps1p.tile([HG, B * W], f32, tag="ps1")
            ps2 = ps2p.tile([HG, B * W], f32, tag="ps2")
            for t, ps in ((0, ps1), (1, ps2)):
                for cp in range(B // 2):
                    dst = ps[:][:, cp * 2 * W:(cp * 2 + 2) * W]
                    center = _rep_ap(gbr[:, cp * 2, t, 1:2],
                                     [[2 * WPAD, 2], [1, HG], [0, 2]])
                    nc.tensor.matmul(dst, u3[:], center,
                                     start=True, stop=False)
                for cp in range(B // 2):
                    dst = ps[:][:, cp * 2 * W:(cp * 2 + 2) * W]
                    shift = _rep_ap(gbr[:, cp * 2, t, 0:1],
                                    [[2 * WPAD, 2], [1, HG], [2, 2]])
                    nc.tensor.matmul(dst, u1[:], shift,
                                     start=False, stop=True)
            g2u_r = ps2[:].rearrange("p (c w) -> p c w", c=B)
            # channel-seam reset for the scan carry
            nc.vector.memset(g2u_r[:, :, 0:1], 0.0)

            # ScalarE: g1u cast to fp16
            g1u = g1sp.tile([HG, B * W], f16, tag="g1u")
            nc.scalar.copy(g1u[:], ps1[:])

            g1ur = g1u[:].rearrange("p (c w) -> p c w", c=B)
            xb = xb2[:].rearrange("p (c q w) -> p c q w", c=B, q=2)[:, :, pstart]
            d = datp.tile([HG, B * W], f16, tag="d")
            dr = d[:].rearrange("p (c w) -> p c w", c=B)

            # d col0 (precomputed b0*s0c)
            nc.gpsimd.tensor_copy(
                dr[:, :, 0:1], d0_all[:, c0:c0 + B].unsqueeze(-1))
            # d = g1u*x in two pieces so the correction subtract (gpsimd)
            # overlaps the big tail multiply on the DVE
            nc.vector.tensor_tensor(
                dr[:, :, 1:K + 2], g1ur[:, :, 1:K + 2], xb[:, :, 1:K + 2],
                Alu.mult)
            qor = qo_all[:].rearrange("p (c w) -> p c w", c=C)
            nc.gpsimd.tensor_tensor(
                dr[:, :, 1:KP], dr[:, :, 1:KP],
                qor[:, c0:c0 + B, 1:KP], Alu.subtract)
            nc.vector.tensor_tensor(
                dr[:, :, K + 2:], g1ur[:, :, K + 2:], xb[:, :, K + 2:],
                Alu.mult)

            # main scan: s[x] = g2u[x]*s[x-1] + d[x]
            ot = outp.tile([HG, B * W], f16, tag="ot")
            nc.vector.tensor_tensor_scan(
                ot[:], ps2[:], d[:], 0.0, Alu.mult, Alu.add)
            nc.sync.dma_start(
                Od[c0:c0 + B, pstart:H:2, :].transpose([1, 0, 2]),
                ot[:].rearrange("p (c w) -> p c w", c=B))


def build():
    nc = bacc.Bacc("TRN2", target_bir_lowering=False, debug=False,
                   num_devices=NCORES)
    Xd = nc.dram_tensor("X", [C, H, W], f16, kind="ExternalInput")
    G12d = nc.dram_tensor("G12", [C, HG, 2, WPAD], f16, kind="ExternalInput")
    Auxd = nc.dram_tensor("AUX", [HG, C * (2 + NG2C + 2)], f16,
                          kind="ExternalInput")
    Ud = {n: nc.dram_tensor(n.upper(), [HG, HG], f16, kind="ExternalInput")
          for n in ("e3", "e1", "o3", "o1")}
    Od = nc.dram_tensor("O", [C, H, W], f16, kind="ExternalOutput")

    with tile.TileContext(nc) as tc:
        with (
            tc.tile_pool(name="const", bufs=1) as constp,
            tc.tile_pool(name="gin", bufs=3) as ginp,
            tc.tile_pool(name="ps1", bufs=2, space="PSUM") as ps1p,
            tc.tile_pool(name="ps2", bufs=2, space="PSUM") as ps2p,
            tc.tile_pool(name="xin", bufs=3) as xinp,
            tc.tile_pool(name="g1s", bufs=4) as g1sp,
            tc.tile_pool(name="dat", bufs=4) as datp,
            tc.tile_pool(name="out", bufs=4) as outp,
        ):
            weights = {}
            for n in ("e3", "e1", "o3", "o1"):
                w = constp.tile([HG, HG], f16, tag=f"u{n}")
                nc.sync.dma_start(w[:], Ud[n][:])
                weights[n] = w
            pools = (ginp, ps1p, ps2p, xinp, g1sp, datp, outp, constp)
            _emit(nc, pools, weights, (Xd, G12d, Auxd, Od))

    nc.compile()
    return nc


_NC = None


def kernel(X, G1, G2, G3=None, **_):
    global _NC
    if _NC is None:
        _NC = build()
    ue, uo = _upsample_mats()
    wmats = {"E3": (3.0 * ue).astype(np.float16),
             "E1": ue.astype(np.float16),
             "O3": (3.0 * uo).astype(np.float16),
             "O1": uo.astype(np.float16)}

    def pad(G):
        return np.concatenate([G[..., :1], G, G[..., -1:]], axis=-1)

    Xh = np.ascontiguousarray(X).astype(np.float16)
    G12h = np.stack([pad(np.asarray(G1)), pad(np.asarray(G2))],
                    axis=3).astype(np.float16)
    # host-packed aux: per (h-row, channel): G1 col0, G2 col0, padded G2
    # head cols 0..NG2C-1, X col0 (even rows), X col0 (odd rows)
    FA = 2 + NG2C + 2
    aux = np.empty((NCORES, HG, C, FA), np.float16)
    aux[..., 0] = G12h[:, :, :, 0, 1].transpose(0, 2, 1)
    aux[..., 1] = G12h[:, :, :, 1, 1].transpose(0, 2, 1)
    aux[..., 2:2 + NG2C] = G12h[:, :, :, 1, 0:NG2C].transpose(0, 2, 1, 3)
    aux[..., FA - 2] = Xh[:, :, 0::2, 0].transpose(0, 2, 1)
    aux[..., FA - 1] = Xh[:, :, 1::2, 0].transpose(0, 2, 1)
    aux = aux.reshape(NCORES, HG, C * FA)

    in_maps = [
        {"X": Xh[k], "G12": np.ascontiguousarray(G12h[k]),
         "AUX": np.ascontiguousarray(aux[k]), **wmats}
        for k in range(NCORES)
    ]
    res = run_bass_kernel_spmd(_NC, in_maps, list(range(NCORES)))
    kernel.last_result = res
    out = np.stack([res.results[k]["O"] for k in range(NCORES)])
    return out.astype(np.float32)
